# revision 1
# baseline (speedup 1.0000x reference)
"""Decoder block (single-head causal attention + GELU FFN) on 8 TRN2 NeuronCores.

Sharding: pure data parallel, no collectives. Core c handles batch b = c//2 and
1024 query tokens of that batch, chosen as two 512-token chunks that balance the
causal-attention workload:
  even cores (half 0): chunks 0 and 3  (rows    0:512  and 1536:2048)
  odd  cores (half 1): chunks 1 and 2  (rows  512:1024 and 1024:1536)
The SPMD program is identical on every core (run_bass_kernel_spmd compiles one
program); all per-core differences are data (which tokens are in x_own, qpos
values that drive on-chip causal-mask generation).

Layout convention: feature-major ("transposed") everywhere on chip. The host
pre-transposes x and all weights, and re-transposes the output, so the kernel
needs no on-chip transposes.

Precision: matmuls run as float32r (full-rate fp32 PE mode) except the
attention probs @ V product, where probs/V are stored fp16 (fp32 PSUM accum).
"""

import numpy as np

D = 1024  # model dim
S = 2048  # sequence length
B = 4  # batch
M = 4096  # FFN dim
CH = 512  # q chunk (slot) size
NDT = D // 128  # 8 d-tiles
N_CORES = 8

_PROGRAM = None  # cached compiled program


def _build_program():
    import sys

    if "/opt/trn_rl_repo" not in sys.path:
        sys.path.insert(0, "/opt/trn_rl_repo")
    import concourse.bass as bass
    import concourse.tile as tile
    import concourse.mybir as mybir
    from concourse import bacc
    from concourse.bass import ts

    dt = mybir.dt
    AF = mybir.ActivationFunctionType
    ALU = mybir.AluOpType
    F32, BF16, F32R, F16 = dt.float32, dt.bfloat16, dt.float32r, dt.float16

    nc = bacc.Bacc("TRN2", target_bir_lowering=False, debug=False)

    # ---------------- DRAM I/O ----------------
    xT = nc.dram_tensor("xT", [D, S], F32R, kind="ExternalInput").ap()
    xoT = nc.dram_tensor("xoT", [D, 2 * CH], F32R, kind="ExternalInput").ap()
    wqT = nc.dram_tensor("wqT", [D, D], F32R, kind="ExternalInput").ap()
    wkT = nc.dram_tensor("wkT", [D, D], F32R, kind="ExternalInput").ap()
    wvT = nc.dram_tensor("wvT", [D, D], F32R, kind="ExternalInput").ap()
    woT = nc.dram_tensor("woT", [D, D], F32R, kind="ExternalInput").ap()
    wfT = nc.dram_tensor("wfT", [D, M], F32R, kind="ExternalInput").ap()
    bq = nc.dram_tensor("bq", [128, D // 128], F32, kind="ExternalInput").ap()
    bk = nc.dram_tensor("bk", [128, D // 128], F32, kind="ExternalInput").ap()
    bo2 = nc.dram_tensor("bo2", [128, D // 128], F32, kind="ExternalInput").ap()
    bfT = nc.dram_tensor("bfT", [128, M // 128], F32, kind="ExternalInput").ap()
    qpos = nc.dram_tensor("qpos", [1, 2 * CH], F32R, kind="ExternalInput").ap()
    iota_kt = nc.dram_tensor("iota_kt", [128, S // 128], F32, kind="ExternalInput").ap()
    ffT = nc.dram_tensor("ffT", [M, 2 * CH], F32, kind="ExternalOutput").ap()

    vS = nc.dram_tensor("vS", [S, D], F16).ap()  # V scratch, token-major, fp16

    NKT = [8, 16]  # k-tiles per slot (slotA: k<1024, slotB: k<2048)

    with tile.TileContext(nc) as tc:
        with (
            tc.tile_pool(name="const", bufs=1) as cpool,
            tc.tile_pool(name="persist", bufs=1) as ppool,
            tc.tile_pool(name="psum", bufs=1, space="PSUM") as pspool,
        ):
            # ---------------- constants ----------------
            ones_col_bf = cpool.tile([128, 1], F16, name="ones_col_bf", tag="ones_col_bf")
            nc.vector.memset(ones_col_bf[:], 1.0)
            ones_row_f = cpool.tile([1, 128], F32, name="ones_row_f", tag="ones_row_f")
            nc.vector.memset(ones_row_f[:], 1.0)
            ones_row = cpool.tile([1, 128], F32R, name="ones_row", tag="ones_row")
            nc.vector.tensor_copy(ones_row[:], ones_row_f[:])
            iota_sb = cpool.tile([128, S // 128], F32, name="iota", tag="iota")
            nc.sync.dma_start(iota_sb[:], iota_kt[:])
            bq_sb = cpool.tile([128, D // 128], F32, name="bq", tag="bq")
            nc.sync.dma_start(bq_sb[:], bq[:])
            bk_sb = cpool.tile([128, D // 128], F32, name="bk", tag="bk")
            nc.sync.dma_start(bk_sb[:], bk[:])
            bo2_sb = cpool.tile([128, D // 128], F32, name="bo2", tag="bo2")
            nc.sync.dma_start(bo2_sb[:], bo2[:])
            bf_sb = cpool.tile([128, M // 128], F32, name="bf", tag="bf")
            nc.sync.dma_start(bf_sb[:], bfT[:])
            qpos_row = cpool.tile([1, 2 * CH], F32R, name="qpos_row", tag="qpos_row")
            nc.sync.dma_start(qpos_row[:], qpos[:])

            # broadcast qpos to 128 partitions via ones outer-product
            qposB = cpool.tile([128, 2 * CH], F32, name="qposB", tag="qposB")
            for i in range(2 * CH // 512):
                bc_ps = pspool.tile([128, 512], F32, name="small", tag="small", bufs=1)
                nc.tensor.matmul(
                    bc_ps[:], ones_row[:], qpos_row[:, ts(i, 512)],
                    start=True, stop=True,
                )
                nc.scalar.activation(qposB[:, ts(i, 512)], bc_ps[:], AF.Copy)

            # ---------------- P1 + P2 ----------------
            attnT = [
                [ppool.tile([128, CH], F32R, name=f"at{dt_}_{c}", tag=f"at{dt_}_{c}") for c in range(2)]
                for dt_ in range(NDT)
            ]
            with tc.tile_pool(name="ktpool", bufs=1) as ktp:
                # K^T stays resident in SBUF for the whole attention phase
                kT = [ktp.tile([128, S], F32R, name=f"kT{i}", tag=f"kT{i}") for i in range(NDT)]

                with tc.tile_pool(name="p1a", bufs=1) as p1a:
                    wk_sb = [p1a.tile([128, D], F32R, name=f"wk{i}", tag=f"wk{i}") for i in range(NDT)]
                    wv_sb = [p1a.tile([128, D], F32R, name=f"wv{i}", tag=f"wv{i}") for i in range(NDT)]
                    for h in range(2):
                        for i in range(NDT):
                            nc.sync.dma_start(
                                wk_sb[i][:, ts(h, 512)], wkT[ts(i, 128), ts(h, 512)]
                            )
                    for tb in range(S // 512):  # K pass: psum -> resident kT
                        xblk = [p1a.tile([128, 512], F32R, name=f"xa{i}", tag=f"xa{i}", bufs=2) for i in range(NDT)]
                        for i in range(NDT):
                            nc.sync.dma_start(xblk[i][:], xT[ts(i, 128), ts(tb, 512)])
                        for ot in range(NDT):
                            ps = pspool.tile([128, 512], F32, name="mm", tag="mm", bufs=3)
                            for i in range(NDT):
                                nc.tensor.matmul(
                                    ps[:], wk_sb[i][:, ts(ot, 128)], xblk[i][:],
                                    start=(i == 0), stop=(i == NDT - 1),
                                )
                            nc.scalar.activation(
                                kT[ot][:, ts(tb, 512)], ps[:], AF.Identity,
                                bias=bk_sb[:, ot : ot + 1],
                            )
                        if tb < 2:  # stagger wv loads behind the x stream
                            for i in range(NDT):
                                nc.sync.dma_start(
                                    wv_sb[i][:, ts(tb, 512)], wvT[ts(i, 128), ts(tb, 512)]
                                )
                    for tb in range(S // 512):  # V pass (re-reads x)
                        xblk = [p1a.tile([128, 512], F32R, name=f"xa{i}", tag=f"xa{i}", bufs=2) for i in range(NDT)]
                        for i in range(NDT):
                            nc.sync.dma_start(xblk[i][:], xT[ts(i, 128), ts(tb, 512)])
                        for tt in range(4):
                            stv = p1a.tile([128, D], F16, name="vstage", tag="vstage", bufs=3)
                            for ob in range(2):
                                ps = pspool.tile([128, 512], F32, name="mm", tag="mm", bufs=3)
                                for i in range(NDT):
                                    nc.tensor.matmul(
                                        ps[:],
                                        xblk[i][:, ts(tt, 128)],
                                        wv_sb[i][:, ts(ob, 512)],
                                        start=(i == 0), stop=(i == NDT - 1),
                                    )
                                nc.scalar.activation(stv[:, ts(ob, 512)], ps[:], AF.Copy)
                            nc.sync.dma_start(vS[ts(tb * 4 + tt, 128), :], stv[:])

                with tc.tile_pool(name="qtpool", bufs=1) as qtp:
                    qT = [
                        [qtp.tile([128, CH], F32R, name=f"qT{dt_}_{qb}", tag=f"qT{dt_}_{qb}") for qb in range(2)]
                        for dt_ in range(NDT)
                    ]
                    with tc.tile_pool(name="p1c", bufs=1) as p1c:
                        wq_sb = [p1c.tile([128, D], F32R, name=f"wq{i}", tag=f"wq{i}") for i in range(NDT)]
                        for h in range(2):
                            for i in range(NDT):
                                nc.sync.dma_start(
                                    wq_sb[i][:, ts(h, 512)], wqT[ts(i, 128), ts(h, 512)]
                                )
                        for qb in range(2):
                            xblk = [p1c.tile([128, 512], F32R, name=f"xc{i}", tag=f"xc{i}", bufs=2) for i in range(NDT)]
                            for i in range(NDT):
                                nc.sync.dma_start(xblk[i][:], xoT[ts(i, 128), ts(qb, 512)])
                            for ot in range(NDT):
                                ps = pspool.tile([128, 512], F32, name="mm", tag="mm", bufs=3)
                                for i in range(NDT):
                                    nc.tensor.matmul(
                                        ps[:], wq_sb[i][:, ts(ot, 128)], xblk[i][:],
                                        start=(i == 0), stop=(i == NDT - 1),
                                    )
                                nc.scalar.activation(
                                    qT[ot][qb][:], ps[:], AF.Identity, bias=bq_sb[:, ot : ot + 1]
                                )

                    # ---------------- P2: attention ----------------
                    with tc.tile_pool(name="p2", bufs=1) as p2:
                        vt = [
                            p2.tile([128, D], F16, name=f"v{k}", tag=f"v{k}", bufs=1)
                            for k in range(16)
                        ]
                        for k in range(16):
                            nc.sync.dma_start(vt[k][:], vS[ts(k, 128), :])
                        for ch in range(2):
                            nkt = NKT[ch]
                            pt = [
                                p2.tile([128, CH], F16, name=f"pt{k}", tag=f"pt{k}", bufs=1)
                                for k in range(nkt)
                            ]
                            dn_ps = pspool.tile([1, CH], F32, name="small", tag="small", bufs=1)
                            for k in range(nkt):
                                ps = pspool.tile([128, CH], F32, name="mm", tag="mm", bufs=3)
                                for i in range(NDT):
                                    nc.tensor.matmul(
                                        ps[:],
                                        kT[i][:, ts(k, 128)],
                                        qT[i][ch][:],
                                        start=(i == 0), stop=(i == NDT - 1),
                                    )
                                masked = (ch == 0) or (k >= 8)
                                if masked:
                                    praw = p2.tile([128, CH], F16, name="praw", tag="praw", bufs=2)
                                    nc.scalar.activation(
                                        praw[:], ps[:], AF.Exp, scale=1.0 / 32.0
                                    )
                                    msk = p2.tile([128, CH], F16, name="msk", tag="msk", bufs=2)
                                    nc.vector.tensor_scalar(
                                        out=msk[:],
                                        in0=qposB[:, ts(ch, CH)],
                                        scalar1=iota_sb[:, k : k + 1],
                                        scalar2=None,
                                        op0=ALU.is_ge,
                                    )
                                    nc.vector.tensor_tensor(
                                        out=pt[k][:], in0=praw[:], in1=msk[:],
                                        op=ALU.mult,
                                    )
                                else:
                                    nc.scalar.activation(
                                        pt[k][:], ps[:], AF.Exp, scale=1.0 / 32.0
                                    )
                                nc.tensor.matmul(
                                    dn_ps[:], ones_col_bf[:], pt[k][:],
                                    start=(k == 0), stop=(k == nkt - 1),
                                )
                            # 1/denom, broadcast to 128 partitions
                            recip = p2.tile([1, CH], F32, name="recip", tag="recip", bufs=2)
                            nc.vector.reciprocal(recip[:], dn_ps[:])
                            recip_r = p2.tile([1, CH], F32R, name="recip_r", tag="recip_r", bufs=2)
                            nc.vector.tensor_copy(recip_r[:], recip[:])
                            rb_ps = pspool.tile([128, CH], F32, name="small", tag="small", bufs=1)
                            nc.tensor.matmul(
                                rb_ps[:], ones_row[:], recip_r[:], start=True, stop=True
                            )
                            recipB = p2.tile([128, CH], F32, name="recipB", tag="recipB", bufs=1)
                            nc.scalar.activation(recipB[:], rb_ps[:], AF.Copy)
                            # attn^T = (P @ V)^T scaled by 1/denom, two 4-bank d passes
                            for half in range(2):
                                for d4 in range(4):
                                    d_ = half * 4 + d4
                                    aps = pspool.tile([128, CH], F32, name=f"at{d4}", tag=f"at{d4}", bufs=1)
                                    for k in range(nkt):
                                        nc.tensor.matmul(
                                            aps[:],
                                            vt[k][:, ts(d_, 128)],
                                            pt[k][:],
                                            start=(k == 0), stop=(k == nkt - 1),
                                        )
                                    nc.vector.tensor_tensor(
                                        out=attnT[d_][ch][:], in0=aps[:], in1=recipB[:],
                                        op=ALU.mult,
                                    )

            # ---------------- P3: output projection ----------------
            with tc.tile_pool(name="p34", bufs=1) as p34:
                outT = [
                    [p34.tile([128, CH], F32R, name=f"oT{dt_}_{c}", tag=f"oT{dt_}_{c}") for c in range(2)]
                    for dt_ in range(NDT)
                ]
                with tc.tile_pool(name="p3", bufs=1) as p3:
                    wo_sb = [p3.tile([128, D], F32R, name=f"wo{i}", tag=f"wo{i}") for i in range(NDT)]
                    for h in range(2):
                        for i in range(NDT):
                            nc.sync.dma_start(
                                wo_sb[i][:, ts(h, 512)], woT[ts(i, 128), ts(h, 512)]
                            )
                    for ch in range(2):
                        for ot in range(NDT):
                            ps = pspool.tile([128, CH], F32, name="mm", tag="mm", bufs=3)
                            for i in range(NDT):
                                nc.tensor.matmul(
                                    ps[:],
                                    wo_sb[i][:, ts(ot, 128)],
                                    attnT[i][ch][:],
                                    start=(i == 0), stop=(i == NDT - 1),
                                )
                            nc.scalar.activation(
                                outT[ot][ch][:], ps[:], AF.Identity, bias=bo2_sb[:, ot : ot + 1]
                            )

                # ---------------- P4: FFN + GELU ----------------
                with tc.tile_pool(name="p4", bufs=1) as p4:
                    for mb in range(M // 512):
                        wfb = [
                            p4.tile([128, 512], F32R, name=f"wf{i}", tag=f"wf{i}", bufs=2)
                            for i in range(NDT)
                        ]
                        for i in range(NDT):
                            nc.sync.dma_start(wfb[i][:], wfT[ts(i, 128), ts(mb, 512)])
                        for mt in range(4):
                            m = mb * 4 + mt
                            for ch in range(2):
                                ps = pspool.tile([128, CH], F32, name="mm", tag="mm", bufs=3)
                                for i in range(NDT):
                                    nc.tensor.matmul(
                                        ps[:],
                                        wfb[i][:, ts(mt, 128)],
                                        outT[i][ch][:],
                                        start=(i == 0), stop=(i == NDT - 1),
                                    )
                                st = p4.tile([128, CH], F32, name="ffstage", tag="ffstage", bufs=4)
                                nc.scalar.activation(
                                    st[:], ps[:], AF.Gelu, bias=bf_sb[:, m : m + 1]
                                )
                                nc.sync.dma_start(ffT[ts(m, 128), ts(ch, CH)], st[:])

    nc.compile()
    return nc


def _get_program():
    global _PROGRAM
    if _PROGRAM is None:
        _PROGRAM = _build_program()
    return _PROGRAM


def _owned_ranges(core):
    """(a0, b0): start rows of the two 512-token chunks core owns."""
    half = core % 2
    if half == 0:
        return 0, 3 * CH  # chunks 0, 3
    return CH, 2 * CH  # chunks 1, 2


def _make_in_maps(x, Wq, bq, Wk, bk, Wv, bv, Wo, bo, Wf, bf):
    f32 = np.float32
    wqT = np.ascontiguousarray(Wq.T, dtype=f32)
    wkT = np.ascontiguousarray(Wk.T, dtype=f32)
    wvT = np.ascontiguousarray(Wv.T, dtype=f32)
    woT = np.ascontiguousarray(Wo.T, dtype=f32)
    wfT = np.ascontiguousarray(Wf.T, dtype=f32)
    bo2 = (Wo.astype(np.float64) @ bv.astype(np.float64) + bo.astype(np.float64))
    bo2 = np.ascontiguousarray(bo2.astype(f32).reshape(D // 128, 128).T)
    bfT = np.ascontiguousarray(bf.reshape(M // 128, 128).T, dtype=f32)
    iota = (
        np.arange(128, dtype=f32)[:, None]
        + 128.0 * np.arange(S // 128, dtype=f32)[None, :]
    )
    shared = {
        "wqT": wqT, "wkT": wkT, "wvT": wvT, "woT": woT, "wfT": wfT,
        "bq": np.ascontiguousarray(bq.reshape(D // 128, 128).T, dtype=f32),
        "bk": np.ascontiguousarray(bk.reshape(D // 128, 128).T, dtype=f32),
        "bo2": bo2,
        "bfT": bfT,
        "iota_kt": np.ascontiguousarray(iota),
    }
    in_maps = []
    for core in range(N_CORES):
        b = core // 2
        a0, b0 = _owned_ranges(core)
        xTb = np.ascontiguousarray(x[b].T, dtype=f32)  # [D, S]
        xoT = np.ascontiguousarray(
            np.concatenate([xTb[:, a0 : a0 + CH], xTb[:, b0 : b0 + CH]], axis=1)
        )
        qp = np.concatenate(
            [np.arange(a0, a0 + CH), np.arange(b0, b0 + CH)]
        ).astype(f32)[None, :]
        in_maps.append(
            {**shared, "xT": xTb, "xoT": xoT, "qpos": np.ascontiguousarray(qp)}
        )
    return in_maps


def _run(inputs, trace=False, trace_cores=None, tmpdir=None):
    import sys

    if "/opt/trn_rl_repo" not in sys.path:
        sys.path.insert(0, "/opt/trn_rl_repo")
    from concourse.bass_utils import run_bass_kernel_spmd

    nc = _get_program()
    in_maps = _make_in_maps(**inputs)
    res = run_bass_kernel_spmd(
        nc, in_maps, list(range(N_CORES)), trace=trace, trace_cores=trace_cores,
        tmpdir=tmpdir,
    )
    out = np.empty((B, S, M), dtype=np.float32)
    for core in range(N_CORES):
        b = core // 2
        a0, b0 = _owned_ranges(core)
        ffT = res.results[core]["ffT"]  # [M, 1024]
        out[b, a0 : a0 + CH] = ffT[:, :CH].T
        out[b, b0 : b0 + CH] = ffT[:, CH:].T
    return out, res


def kernel(**inputs):
    out, _ = _run(inputs)
    return out



# revision 4
# speedup vs baseline: 1.2511x; 1.2511x over previous
"""Decoder block (single-head causal attention + GELU FFN) on 8 TRN2 NeuronCores.

Sharding: pure data parallel, no collectives. Core c handles batch b = c//2 and
1024 query tokens of that batch, chosen as two 512-token chunks that balance the
causal-attention workload:
  even cores (half 0): chunks 0 and 3  (rows    0:512  and 1536:2048)
  odd  cores (half 1): chunks 1 and 2  (rows  512:1024 and 1024:1536)
The SPMD program is identical on every core; all per-core differences are data
(which tokens are in xoT, qpos values that drive on-chip causal-mask generation).

v3:
  - All matmul operands bf16 (fp32 PSUM accumulation); DMA and SBUF halved.
  - Wo folded into Wv on the host (softmax rows sum to 1), its bias chain
    folded into the FFN bias. The output-projection phase disappears.
  - Single x stream computes K and V' per token-block pair; paired matmul
    chains share the stationary operand (wk across the tb pair, xblk across
    the wv halves, wq across the q chunks, wf across the two out chunks).
  - V' stays in SBUF; softmax denominator accumulated on the vector engine;
    reciprocal broadcast matmuls scheduled between PV chains (after the
    recipB write — program order defines tile dependencies).
  - ffT output in bf16 (host casts back to f32).
"""

import numpy as np

D = 1024  # model dim
S = 2048  # sequence length
B = 4  # batch
M = 4096  # FFN dim
CH = 512  # q chunk (slot) size
NDT = D // 128  # 8 d-tiles
N_CORES = 8

_PROGRAM = None  # cached compiled program


def _build_program():
    import sys

    if "/opt/trn_rl_repo" not in sys.path:
        sys.path.insert(0, "/opt/trn_rl_repo")
    import concourse.bass as bass
    import concourse.tile as tile
    import concourse.mybir as mybir
    from concourse import bacc
    from concourse.bass import ts

    dt = mybir.dt
    AF = mybir.ActivationFunctionType
    ALU = mybir.AluOpType
    F32, BF16, F32R, F16 = dt.float32, dt.bfloat16, dt.float32r, dt.float16

    nc = bacc.Bacc("TRN2", target_bir_lowering=False, debug=False)

    # ---------------- DRAM I/O ----------------
    xT = nc.dram_tensor("xT", [D, S], BF16, kind="ExternalInput").ap()
    xoT = nc.dram_tensor("xoT", [D, 2 * CH], BF16, kind="ExternalInput").ap()
    wqT = nc.dram_tensor("wqT", [D, D], BF16, kind="ExternalInput").ap()
    wkT = nc.dram_tensor("wkT", [D, D], BF16, kind="ExternalInput").ap()
    wvT = nc.dram_tensor("wvT", [D, D], BF16, kind="ExternalInput").ap()  # (Wo@Wv).T
    wfT = nc.dram_tensor("wfT", [D, M], BF16, kind="ExternalInput").ap()
    bq = nc.dram_tensor("bq", [128, D // 128], F32, kind="ExternalInput").ap()
    bk = nc.dram_tensor("bk", [128, D // 128], F32, kind="ExternalInput").ap()
    bfT = nc.dram_tensor("bfT", [128, M // 128], F32, kind="ExternalInput").ap()
    qpos = nc.dram_tensor("qpos", [1, 2 * CH], F32R, kind="ExternalInput").ap()
    iota_kt = nc.dram_tensor("iota_kt", [128, S // 128], F32, kind="ExternalInput").ap()
    ffT = nc.dram_tensor("ffT", [M, 2 * CH], BF16, kind="ExternalOutput").ap()

    NKT = [8, 16]  # k-tiles per slot (slotA: k<1024, slotB: k<2048)

    with tile.TileContext(nc) as tc:
        with (
            tc.tile_pool(name="const", bufs=1) as cpool,
            tc.tile_pool(name="psum", bufs=1, space="PSUM") as pspool,
        ):
            # ---------------- constants ----------------
            ones_col_r = cpool.tile([128, 1], F32R, name="ones_col_r", tag="ones_col_r")
            ones_row_f = cpool.tile([1, 128], F32, name="ones_row_f", tag="ones_row_f")
            nc.vector.memset(ones_row_f[:], 1.0)
            ones_row = cpool.tile([1, 128], F32R, name="ones_row", tag="ones_row")
            nc.vector.tensor_copy(ones_row[:], ones_row_f[:])
            ones_col_f = cpool.tile([128, 1], F32, name="ones_col_f", tag="ones_col_f")
            nc.vector.memset(ones_col_f[:], 1.0)
            nc.vector.tensor_copy(ones_col_r[:], ones_col_f[:])
            iota_sb = cpool.tile([128, S // 128], F32, name="iota", tag="iota")
            nc.sync.dma_start(iota_sb[:], iota_kt[:])
            bq_sb = cpool.tile([128, D // 128], F32, name="bq", tag="bq")
            nc.sync.dma_start(bq_sb[:], bq[:])
            bk_sb = cpool.tile([128, D // 128], F32, name="bk", tag="bk")
            nc.sync.dma_start(bk_sb[:], bk[:])
            bf_sb = cpool.tile([128, M // 128], F32, name="bf", tag="bf")
            nc.sync.dma_start(bf_sb[:], bfT[:])
            qpos_row = cpool.tile([1, 2 * CH], F32R, name="qpos_row", tag="qpos_row")
            nc.sync.dma_start(qpos_row[:], qpos[:])

            # broadcast qpos to 128 partitions via ones outer-product
            qposB = cpool.tile([128, 2 * CH], F32, name="qposB", tag="qposB")
            for i in range(2 * CH // 512):
                bc_ps = pspool.tile([128, 512], F32, name="small", tag="small", bufs=1)
                nc.tensor.matmul(
                    bc_ps[:], ones_row[:], qpos_row[:, ts(i, 512)],
                    start=True, stop=True,
                )
                nc.scalar.activation(qposB[:, ts(i, 512)], bc_ps[:], AF.Copy)

            # attention output (normalized, bf16) — lives P2..P4
            outT = [
                [cpool.tile([128, CH], BF16, name=f"oT{dt_}_{c}", tag=f"oT{dt_}_{c}") for c in range(2)]
                for dt_ in range(NDT)
            ]

            with tc.tile_pool(name="ktpool", bufs=1) as ktp:
                # K^T (bf16) and V' (fp16, token-major) stay resident through P2
                kT = [ktp.tile([128, S], BF16, name=f"kT{i}", tag=f"kT{i}") for i in range(NDT)]
                vt = [ktp.tile([128, D], F16, name=f"v{k}", tag=f"v{k}") for k in range(16)]

                with tc.tile_pool(name="qtpool", bufs=1) as qtp:
                    qT = [
                        [qtp.tile([128, CH], BF16, name=f"qT{dt_}_{qb}", tag=f"qT{dt_}_{qb}") for qb in range(2)]
                        for dt_ in range(NDT)
                    ]

                    # ---------------- P1: K / V' / Q projections ----------------
                    with tc.tile_pool(name="p1", bufs=1) as p1:
                        wk_sb = [p1.tile([128, D], BF16, name=f"wk{i}", tag=f"wk{i}") for i in range(NDT)]
                        wv_sb = [p1.tile([128, D], BF16, name=f"wv{i}", tag=f"wv{i}") for i in range(NDT)]
                        wq_sb = [p1.tile([128, D], BF16, name=f"wq{i}", tag=f"wq{i}") for i in range(NDT)]
                        xo_sb = [p1.tile([128, 2 * CH], BF16, name=f"xo{i}", tag=f"xo{i}") for i in range(NDT)]
                        # startup order: wk h0, then first x block, then wk h1
                        for i in range(NDT):
                            nc.sync.dma_start(wk_sb[i][:, ts(0, 512)], wkT[ts(i, 128), ts(0, 512)])
                        # merged K+V pass over one x stream, tb pairs
                        for p in range(2):
                            xblk = [
                                [
                                    p1.tile([128, 512], BF16, name=f"xa{i}_{j}", tag=f"xa{i}_{j}", bufs=2)
                                    for i in range(NDT)
                                ]
                                for j in range(2)
                            ]
                            for i in range(NDT):
                                nc.sync.dma_start(xblk[0][i][:], xT[ts(i, 128), ts(2 * p, 512)])
                            if p == 0:
                                for i in range(NDT):
                                    nc.sync.dma_start(
                                        wk_sb[i][:, ts(1, 512)], wkT[ts(i, 128), ts(1, 512)]
                                    )
                            for i in range(NDT):
                                nc.sync.dma_start(xblk[1][i][:], xT[ts(i, 128), ts(2 * p + 1, 512)])
                            # stagger remaining loads behind the x stream
                            if p == 0:
                                for h in range(2):
                                    for i in range(NDT):
                                        nc.sync.dma_start(
                                            wv_sb[i][:, ts(h, 512)], wvT[ts(i, 128), ts(h, 512)]
                                        )
                            else:
                                for h in range(2):
                                    for i in range(NDT):
                                        nc.sync.dma_start(
                                            wq_sb[i][:, ts(h, 512)], wqT[ts(i, 128), ts(h, 512)]
                                        )
                                for h in range(2):
                                    for i in range(NDT):
                                        nc.sync.dma_start(
                                            xo_sb[i][:, ts(h, 512)], xoT[ts(i, 128), ts(h, 512)]
                                        )
                            # K chains: stationary wk slice shared across the tb pair
                            for ot in range(NDT):
                                psA = pspool.tile([128, 512], F32, name="mm", tag="mm", bufs=2)
                                psB = pspool.tile([128, 512], F32, name=f"at{ot % 2}", tag=f"at{ot % 2}", bufs=1)
                                for i in range(NDT):
                                    nc.tensor.matmul(
                                        psA[:], wk_sb[i][:, ts(ot, 128)], xblk[0][i][:],
                                        start=(i == 0), stop=(i == NDT - 1),
                                    )
                                    nc.tensor.matmul(
                                        psB[:], wk_sb[i][:, ts(ot, 128)], xblk[1][i][:],
                                        start=(i == 0), stop=(i == NDT - 1),
                                    )
                                nc.scalar.activation(
                                    kT[ot][:, ts(2 * p, 512)], psA[:], AF.Identity,
                                    bias=bk_sb[:, ot : ot + 1],
                                )
                                nc.scalar.activation(
                                    kT[ot][:, ts(2 * p + 1, 512)], psB[:], AF.Identity,
                                    bias=bk_sb[:, ot : ot + 1],
                                )
                            # V chains: stationary x token-tile shared across wv halves
                            for j in range(2):
                                tb = 2 * p + j
                                for tt in range(4):
                                    k = tb * 4 + tt
                                    psA = pspool.tile([128, 512], F32, name="mm", tag="mm", bufs=2)
                                    psB = pspool.tile([128, 512], F32, name=f"at{tt % 2}", tag=f"at{tt % 2}", bufs=1)
                                    for i in range(NDT):
                                        nc.tensor.matmul(
                                            psA[:], xblk[j][i][:, ts(tt, 128)], wv_sb[i][:, ts(0, 512)],
                                            start=(i == 0), stop=(i == NDT - 1),
                                        )
                                        nc.tensor.matmul(
                                            psB[:], xblk[j][i][:, ts(tt, 128)], wv_sb[i][:, ts(1, 512)],
                                            start=(i == 0), stop=(i == NDT - 1),
                                        )
                                    nc.scalar.activation(vt[k][:, ts(0, 512)], psA[:], AF.Copy)
                                    nc.scalar.activation(vt[k][:, ts(1, 512)], psB[:], AF.Copy)
                        # Q pass: stationary wq slice shared across the q chunks
                        for ot in range(NDT):
                            psA = pspool.tile([128, 512], F32, name="mm", tag="mm", bufs=2)
                            psB = pspool.tile([128, 512], F32, name=f"at{ot % 2}", tag=f"at{ot % 2}", bufs=1)
                            for i in range(NDT):
                                nc.tensor.matmul(
                                    psA[:], wq_sb[i][:, ts(ot, 128)], xo_sb[i][:, ts(0, 512)],
                                    start=(i == 0), stop=(i == NDT - 1),
                                )
                                nc.tensor.matmul(
                                    psB[:], wq_sb[i][:, ts(ot, 128)], xo_sb[i][:, ts(1, 512)],
                                    start=(i == 0), stop=(i == NDT - 1),
                                )
                            nc.scalar.activation(
                                qT[ot][0][:], psA[:], AF.Identity, bias=bq_sb[:, ot : ot + 1]
                            )
                            nc.scalar.activation(
                                qT[ot][1][:], psB[:], AF.Identity, bias=bq_sb[:, ot : ot + 1]
                            )

                    # ---------------- P2: attention ----------------
                    with tc.tile_pool(name="p2", bufs=1) as p2:
                        for ch in range(2):
                            nkt = NKT[ch]
                            pt = [
                                p2.tile([128, CH], F16, name=f"pt{k}", tag=f"pt{k}", bufs=1)
                                for k in range(nkt)
                            ]
                            ptsum = p2.tile([128, CH], F32R, name="ptsum", tag="ptsum", bufs=2)
                            for k in range(nkt):
                                ps = pspool.tile([128, CH], F32, name="mm", tag="mm", bufs=2)
                                for i in range(NDT):
                                    nc.tensor.matmul(
                                        ps[:],
                                        kT[i][:, ts(k, 128)],
                                        qT[i][ch][:],
                                        start=(i == 0), stop=(i == NDT - 1),
                                    )
                                masked = (ch == 0) or (k >= 8)
                                if masked:
                                    praw = p2.tile([128, CH], F16, name="praw", tag="praw", bufs=2)
                                    nc.scalar.activation(
                                        praw[:], ps[:], AF.Exp, scale=1.0 / 32.0
                                    )
                                    msk = p2.tile([128, CH], F16, name="msk", tag="msk", bufs=2)
                                    nc.vector.tensor_scalar(
                                        out=msk[:],
                                        in0=qposB[:, ts(ch, CH)],
                                        scalar1=iota_sb[:, k : k + 1],
                                        scalar2=None,
                                        op0=ALU.is_ge,
                                    )
                                    nc.vector.tensor_tensor(
                                        out=pt[k][:], in0=praw[:], in1=msk[:],
                                        op=ALU.mult,
                                    )
                                else:
                                    nc.scalar.activation(
                                        pt[k][:], ps[:], AF.Exp, scale=1.0 / 32.0
                                    )
                                # denominator accumulation on DVE (off the PE)
                                if k == 0:
                                    nc.vector.tensor_copy(ptsum[:], pt[0][:])
                                else:
                                    nc.vector.tensor_tensor(
                                        out=ptsum[:], in0=ptsum[:], in1=pt[k][:],
                                        op=ALU.add,
                                    )

                            recip = p2.tile([1, CH], F32, name="recip", tag="recip", bufs=2)
                            recip_r = p2.tile([1, CH], F32R, name="recip_r", tag="recip_r", bufs=2)
                            recipB = p2.tile([128, CH], F32, name="recipB", tag="recipB", bufs=2)
                            aps = {}

                            def pv_mm(d_):
                                aps[d_] = pspool.tile(
                                    [128, CH], F32, name=f"at{d_ % 5}", tag=f"at{d_ % 5}", bufs=1
                                )
                                for k in range(nkt):
                                    nc.tensor.matmul(
                                        aps[d_][:],
                                        vt[k][:, ts(d_, 128)],
                                        pt[k][:],
                                        start=(k == 0), stop=(k == nkt - 1),
                                    )

                            def pv_mult(d_):
                                nc.vector.tensor_tensor(
                                    out=outT[d_][ch][:], in0=aps[d_][:], in1=recipB[:],
                                    op=ALU.mult,
                                )

                            # PE order: PV0 | reduce | PV1-3 | broadcast | PV4-7.
                            # All outT multiplies are emitted AFTER the recipB
                            # write (program order defines tile dependencies).
                            pv_mm(0)
                            dn_ps = pspool.tile([1, CH], F32, name="small", tag="small", bufs=1)
                            nc.tensor.matmul(
                                dn_ps[:], ones_col_r[:], ptsum[:], start=True, stop=True
                            )
                            nc.vector.reciprocal(recip[:], dn_ps[:])
                            nc.vector.tensor_copy(recip_r[:], recip[:])
                            for d_ in range(1, 4):
                                pv_mm(d_)
                            rb_ps = pspool.tile([128, CH], F32, name="small", tag="small", bufs=1)
                            nc.tensor.matmul(
                                rb_ps[:], ones_row[:], recip_r[:], start=True, stop=True
                            )
                            nc.scalar.activation(recipB[:], rb_ps[:], AF.Copy)
                            for d_ in range(4):
                                pv_mult(d_)
                            for d_ in range(4, NDT):
                                pv_mm(d_)
                                pv_mult(d_)

            # ---------------- P4: FFN + GELU ----------------
            with tc.tile_pool(name="p4", bufs=1) as p4:
                for mb in range(M // 512):
                    wfb = [
                        p4.tile([128, 512], BF16, name=f"wf{i}", tag=f"wf{i}", bufs=2)
                        for i in range(NDT)
                    ]
                    for i in range(NDT):
                        nc.sync.dma_start(wfb[i][:], wfT[ts(i, 128), ts(mb, 512)])
                    for mt in range(4):
                        m = mb * 4 + mt
                        psA = pspool.tile([128, CH], F32, name="mm", tag="mm", bufs=2)
                        psB = pspool.tile([128, CH], F32, name=f"at{mt % 2}", tag=f"at{mt % 2}", bufs=1)
                        for i in range(NDT):
                            nc.tensor.matmul(
                                psA[:], wfb[i][:, ts(mt, 128)], outT[i][0][:],
                                start=(i == 0), stop=(i == NDT - 1),
                            )
                            nc.tensor.matmul(
                                psB[:], wfb[i][:, ts(mt, 128)], outT[i][1][:],
                                start=(i == 0), stop=(i == NDT - 1),
                            )
                        stA = p4.tile([128, CH], BF16, name="ffstage", tag="ffstage", bufs=4)
                        nc.scalar.activation(stA[:], psA[:], AF.Gelu, bias=bf_sb[:, m : m + 1])
                        nc.sync.dma_start(ffT[ts(m, 128), ts(0, CH)], stA[:])
                        stB = p4.tile([128, CH], BF16, name="ffstage", tag="ffstage", bufs=4)
                        nc.scalar.activation(stB[:], psB[:], AF.Gelu, bias=bf_sb[:, m : m + 1])
                        nc.sync.dma_start(ffT[ts(m, 128), ts(1, CH)], stB[:])

    nc.compile()
    return nc


def _get_program():
    global _PROGRAM
    if _PROGRAM is None:
        _PROGRAM = _build_program()
    return _PROGRAM


def _owned_ranges(core):
    """(a0, b0): start rows of the two 512-token chunks core owns."""
    half = core % 2
    if half == 0:
        return 0, 3 * CH  # chunks 0, 3
    return CH, 2 * CH  # chunks 1, 2


def _make_in_maps(x, Wq, bq, Wk, bk, Wv, bv, Wo, bo, Wf, bf):
    import ml_dtypes

    f32 = np.float32
    bf16 = ml_dtypes.bfloat16
    f64 = np.float64
    wqT = np.ascontiguousarray(Wq.T, dtype=bf16)
    wkT = np.ascontiguousarray(Wk.T, dtype=bf16)
    Wvp = (np.asarray(Wo, f64) @ np.asarray(Wv, f64))  # folded V' weight
    wvT = np.ascontiguousarray(Wvp.T.astype(f32), dtype=bf16)
    wfT = np.ascontiguousarray(Wf.T, dtype=bf16)
    # bo2 = Wo@bv + bo folded into the FFN bias: bf2 = Wf@bo2 + bf
    bo2 = np.asarray(Wo, f64) @ np.asarray(bv, f64) + np.asarray(bo, f64)
    bf2 = np.asarray(Wf, f64) @ bo2 + np.asarray(bf, f64)
    bfT = np.ascontiguousarray(bf2.astype(f32).reshape(M // 128, 128).T)
    iota = (
        np.arange(128, dtype=f32)[:, None]
        + 128.0 * np.arange(S // 128, dtype=f32)[None, :]
    )
    shared = {
        "wqT": wqT, "wkT": wkT, "wvT": wvT, "wfT": wfT,
        "bq": np.ascontiguousarray(bq.reshape(D // 128, 128).T, dtype=f32),
        "bk": np.ascontiguousarray(bk.reshape(D // 128, 128).T, dtype=f32),
        "bfT": bfT,
        "iota_kt": np.ascontiguousarray(iota),
    }
    in_maps = []
    for core in range(N_CORES):
        b = core // 2
        a0, b0 = _owned_ranges(core)
        xTb = np.ascontiguousarray(x[b].T, dtype=bf16)  # [D, S]
        xoT = np.ascontiguousarray(
            np.concatenate([xTb[:, a0 : a0 + CH], xTb[:, b0 : b0 + CH]], axis=1)
        )
        qp = np.concatenate(
            [np.arange(a0, a0 + CH), np.arange(b0, b0 + CH)]
        ).astype(f32)[None, :]
        in_maps.append(
            {**shared, "xT": xTb, "xoT": xoT, "qpos": np.ascontiguousarray(qp)}
        )
    return in_maps


def _run(inputs, trace=False, trace_cores=None, tmpdir=None):
    import sys

    if "/opt/trn_rl_repo" not in sys.path:
        sys.path.insert(0, "/opt/trn_rl_repo")
    from concourse.bass_utils import run_bass_kernel_spmd

    nc = _get_program()
    in_maps = _make_in_maps(**inputs)
    res = run_bass_kernel_spmd(
        nc, in_maps, list(range(N_CORES)), trace=trace, trace_cores=trace_cores,
        tmpdir=tmpdir,
    )
    out = np.empty((B, S, M), dtype=np.float32)
    for core in range(N_CORES):
        b = core // 2
        a0, b0 = _owned_ranges(core)
        ffT = np.asarray(res.results[core]["ffT"], dtype=np.float32)  # [M, 1024]
        out[b, a0 : a0 + CH] = ffT[:, :CH].T
        out[b, b0 : b0 + CH] = ffT[:, CH:].T
    return out, res


def kernel(**inputs):
    out, _ = _run(inputs)
    return out


# revision 5
# speedup vs baseline: 1.2560x; 1.0039x over previous
"""Decoder block (single-head causal attention + GELU FFN) on 8 TRN2 NeuronCores.

Sharding: pure data parallel, no collectives. Core c handles batch b = c//2 and
1024 query tokens of that batch, chosen as two 512-token chunks that balance the
causal-attention workload:
  even cores (half 0): chunks 0 and 3  (rows    0:512  and 1536:2048)
  odd  cores (half 1): chunks 1 and 2  (rows  512:1024 and 1024:1536)
The SPMD program is identical on every core; all per-core differences are data
(which tokens are in xoT, qpos values that drive on-chip causal-mask generation).

v4:
  - All matmul operands bf16 (fp32 PSUM accumulation); DMA and SBUF halved.
  - Wo folded into Wv on the host (softmax rows sum to 1), its bias chain
    folded into the FFN bias. The output-projection phase disappears.
  - Single x stream computes K and V' per token-block pair; paired matmul
    chains share the stationary operand (wk across the tb pair, xblk across
    the wv halves, wq across the q chunks, wf across the two out chunks).
  - V' stays in SBUF; softmax denominator accumulated on the vector engine;
    reciprocal broadcast matmuls scheduled between PV chains (after the
    recipB write — program order defines tile dependencies).
  - ffT output in bf16 (host casts back to f32).
"""

import numpy as np

D = 1024  # model dim
S = 2048  # sequence length
B = 4  # batch
M = 4096  # FFN dim
CH = 512  # q chunk (slot) size
NDT = D // 128  # 8 d-tiles
N_CORES = 8

_PROGRAM = None  # cached compiled program


def _build_program():
    import sys

    if "/opt/trn_rl_repo" not in sys.path:
        sys.path.insert(0, "/opt/trn_rl_repo")
    import concourse.bass as bass
    import concourse.tile as tile
    import concourse.mybir as mybir
    from concourse import bacc
    from concourse.bass import ts

    dt = mybir.dt
    AF = mybir.ActivationFunctionType
    ALU = mybir.AluOpType
    F32, BF16, F32R, F16 = dt.float32, dt.bfloat16, dt.float32r, dt.float16

    nc = bacc.Bacc("TRN2", target_bir_lowering=False, debug=False)

    # ---------------- DRAM I/O ----------------
    xT = nc.dram_tensor("xT", [D, S], BF16, kind="ExternalInput").ap()
    xoT = nc.dram_tensor("xoT", [D, 2 * CH], BF16, kind="ExternalInput").ap()
    wqT = nc.dram_tensor("wqT", [D, D], BF16, kind="ExternalInput").ap()
    wkT = nc.dram_tensor("wkT", [D, D], BF16, kind="ExternalInput").ap()
    wvT = nc.dram_tensor("wvT", [D, D], BF16, kind="ExternalInput").ap()  # (Wo@Wv).T
    wfT = nc.dram_tensor("wfT", [D, M], BF16, kind="ExternalInput").ap()
    bq = nc.dram_tensor("bq", [128, D // 128], F32, kind="ExternalInput").ap()
    bk = nc.dram_tensor("bk", [128, D // 128], F32, kind="ExternalInput").ap()
    bfT = nc.dram_tensor("bfT", [128, M // 128], F32, kind="ExternalInput").ap()
    qpos = nc.dram_tensor("qpos", [1, 2 * CH], F32R, kind="ExternalInput").ap()
    iota_kt = nc.dram_tensor("iota_kt", [128, S // 128], F32, kind="ExternalInput").ap()
    ffT = nc.dram_tensor("ffT", [M, 2 * CH], BF16, kind="ExternalOutput").ap()

    NKT = [8, 16]  # k-tiles per slot (slotA: k<1024, slotB: k<2048)

    with tile.TileContext(nc) as tc:
        with (
            tc.tile_pool(name="const", bufs=1) as cpool,
            tc.tile_pool(name="psum", bufs=1, space="PSUM") as pspool,
        ):
            # ---------------- constants ----------------
            ones_col_r = cpool.tile([128, 1], F32R, name="ones_col_r", tag="ones_col_r")
            ones_row_f = cpool.tile([1, 128], F32, name="ones_row_f", tag="ones_row_f")
            nc.vector.memset(ones_row_f[:], 1.0)
            ones_row = cpool.tile([1, 128], F32R, name="ones_row", tag="ones_row")
            nc.vector.tensor_copy(ones_row[:], ones_row_f[:])
            ones_col_f = cpool.tile([128, 1], F32, name="ones_col_f", tag="ones_col_f")
            nc.vector.memset(ones_col_f[:], 1.0)
            nc.vector.tensor_copy(ones_col_r[:], ones_col_f[:])
            iota_sb = cpool.tile([128, S // 128], F32, name="iota", tag="iota")
            nc.sync.dma_start(iota_sb[:], iota_kt[:])
            bq_sb = cpool.tile([128, D // 128], F32, name="bq", tag="bq")
            nc.sync.dma_start(bq_sb[:], bq[:])
            bk_sb = cpool.tile([128, D // 128], F32, name="bk", tag="bk")
            nc.sync.dma_start(bk_sb[:], bk[:])
            bf_sb = cpool.tile([128, M // 128], F32, name="bf", tag="bf")
            nc.sync.dma_start(bf_sb[:], bfT[:])
            qpos_row = cpool.tile([1, 2 * CH], F32R, name="qpos_row", tag="qpos_row")
            nc.sync.dma_start(qpos_row[:], qpos[:])

            # broadcast qpos to 128 partitions via ones outer-product
            qposB = cpool.tile([128, 2 * CH], F32, name="qposB", tag="qposB")
            for i in range(2 * CH // 512):
                bc_ps = pspool.tile([128, 512], F32, name="small", tag="small", bufs=1)
                nc.tensor.matmul(
                    bc_ps[:], ones_row[:], qpos_row[:, ts(i, 512)],
                    start=True, stop=True,
                )
                nc.scalar.activation(qposB[:, ts(i, 512)], bc_ps[:], AF.Copy)

            # attention output (normalized, bf16) — lives P2..P4
            outT = [
                [cpool.tile([128, CH], BF16, name=f"oT{dt_}_{c}", tag=f"oT{dt_}_{c}") for c in range(2)]
                for dt_ in range(NDT)
            ]

            with tc.tile_pool(name="ktpool", bufs=1) as ktp:
                # K^T (bf16) and V' (fp16, token-major) stay resident through P2
                kT = [ktp.tile([128, S], BF16, name=f"kT{i}", tag=f"kT{i}") for i in range(NDT)]
                vt = [ktp.tile([128, D], F16, name=f"v{k}", tag=f"v{k}") for k in range(16)]

                with tc.tile_pool(name="qtpool", bufs=1) as qtp:
                    qT = [
                        [qtp.tile([128, CH], BF16, name=f"qT{dt_}_{qb}", tag=f"qT{dt_}_{qb}") for qb in range(2)]
                        for dt_ in range(NDT)
                    ]

                    # ---------------- P1: K / V' / Q projections ----------------
                    with tc.tile_pool(name="p1", bufs=1) as p1:
                        wk_sb = [p1.tile([128, D], BF16, name=f"wk{i}", tag=f"wk{i}") for i in range(NDT)]
                        wv_sb = [p1.tile([128, D], BF16, name=f"wv{i}", tag=f"wv{i}") for i in range(NDT)]
                        wq_sb = [p1.tile([128, D], BF16, name=f"wq{i}", tag=f"wq{i}") for i in range(NDT)]
                        xo_sb = [p1.tile([128, 2 * CH], BF16, name=f"xo{i}", tag=f"xo{i}") for i in range(NDT)]
                        # startup order: wk h0, then first x block, then wk h1
                        for i in range(NDT):
                            nc.sync.dma_start(wk_sb[i][:, ts(0, 512)], wkT[ts(i, 128), ts(0, 512)])
                        # merged K+V pass over one x stream, tb pairs
                        for p in range(2):
                            xblk = [
                                [
                                    p1.tile([128, 512], BF16, name=f"xa{i}_{j}", tag=f"xa{i}_{j}", bufs=2)
                                    for i in range(NDT)
                                ]
                                for j in range(2)
                            ]
                            for i in range(NDT):
                                nc.sync.dma_start(xblk[0][i][:], xT[ts(i, 128), ts(2 * p, 512)])
                            for i in range(NDT):
                                nc.sync.dma_start(xblk[1][i][:], xT[ts(i, 128), ts(2 * p + 1, 512)])
                            if p == 0:
                                for i in range(NDT):
                                    nc.sync.dma_start(
                                        wk_sb[i][:, ts(1, 512)], wkT[ts(i, 128), ts(1, 512)]
                                    )
                            # stagger remaining loads behind the x stream
                            if p == 0:
                                for h in range(2):
                                    for i in range(NDT):
                                        nc.sync.dma_start(
                                            wv_sb[i][:, ts(h, 512)], wvT[ts(i, 128), ts(h, 512)]
                                        )
                            else:
                                for h in range(2):
                                    for i in range(NDT):
                                        nc.sync.dma_start(
                                            wq_sb[i][:, ts(h, 512)], wqT[ts(i, 128), ts(h, 512)]
                                        )
                                for h in range(2):
                                    for i in range(NDT):
                                        nc.sync.dma_start(
                                            xo_sb[i][:, ts(h, 512)], xoT[ts(i, 128), ts(h, 512)]
                                        )
                            # K chains: stationary wk slice shared across the tb pair
                            for ot in range(NDT):
                                psA = pspool.tile([128, 512], F32, name="mm", tag="mm", bufs=2)
                                psB = pspool.tile([128, 512], F32, name=f"at{ot % 2}", tag=f"at{ot % 2}", bufs=1)
                                for i in range(NDT):
                                    nc.tensor.matmul(
                                        psA[:], wk_sb[i][:, ts(ot, 128)], xblk[0][i][:],
                                        start=(i == 0), stop=(i == NDT - 1),
                                    )
                                    nc.tensor.matmul(
                                        psB[:], wk_sb[i][:, ts(ot, 128)], xblk[1][i][:],
                                        start=(i == 0), stop=(i == NDT - 1),
                                    )
                                nc.scalar.activation(
                                    kT[ot][:, ts(2 * p, 512)], psA[:], AF.Identity,
                                    bias=bk_sb[:, ot : ot + 1],
                                )
                                nc.scalar.activation(
                                    kT[ot][:, ts(2 * p + 1, 512)], psB[:], AF.Identity,
                                    bias=bk_sb[:, ot : ot + 1],
                                )
                            # V chains: stationary x token-tile shared across wv halves
                            for j in range(2):
                                tb = 2 * p + j
                                for tt in range(4):
                                    k = tb * 4 + tt
                                    psA = pspool.tile([128, 512], F32, name="mm", tag="mm", bufs=2)
                                    psB = pspool.tile([128, 512], F32, name=f"at{tt % 2}", tag=f"at{tt % 2}", bufs=1)
                                    for i in range(NDT):
                                        nc.tensor.matmul(
                                            psA[:], xblk[j][i][:, ts(tt, 128)], wv_sb[i][:, ts(0, 512)],
                                            start=(i == 0), stop=(i == NDT - 1),
                                        )
                                        nc.tensor.matmul(
                                            psB[:], xblk[j][i][:, ts(tt, 128)], wv_sb[i][:, ts(1, 512)],
                                            start=(i == 0), stop=(i == NDT - 1),
                                        )
                                    nc.scalar.activation(vt[k][:, ts(0, 512)], psA[:], AF.Copy)
                                    nc.scalar.activation(vt[k][:, ts(1, 512)], psB[:], AF.Copy)
                        # Q pass: stationary wq slice shared across the q chunks
                        for ot in range(NDT):
                            psA = pspool.tile([128, 512], F32, name="mm", tag="mm", bufs=2)
                            psB = pspool.tile([128, 512], F32, name=f"at{ot % 2}", tag=f"at{ot % 2}", bufs=1)
                            for i in range(NDT):
                                nc.tensor.matmul(
                                    psA[:], wq_sb[i][:, ts(ot, 128)], xo_sb[i][:, ts(0, 512)],
                                    start=(i == 0), stop=(i == NDT - 1),
                                )
                                nc.tensor.matmul(
                                    psB[:], wq_sb[i][:, ts(ot, 128)], xo_sb[i][:, ts(1, 512)],
                                    start=(i == 0), stop=(i == NDT - 1),
                                )
                            nc.scalar.activation(
                                qT[ot][0][:], psA[:], AF.Identity, bias=bq_sb[:, ot : ot + 1]
                            )
                            nc.scalar.activation(
                                qT[ot][1][:], psB[:], AF.Identity, bias=bq_sb[:, ot : ot + 1]
                            )

                    # ---------------- P2: attention ----------------
                    with tc.tile_pool(name="p2", bufs=1) as p2:
                        for ch in (1, 0):
                            nkt = NKT[ch]
                            pt = [
                                p2.tile([128, CH], F16, name=f"pt{k}", tag=f"pt{k}", bufs=1)
                                for k in range(nkt)
                            ]
                            ptsum = p2.tile([128, CH], F32R, name="ptsum", tag="ptsum", bufs=2)
                            for k in range(nkt):
                                ps = pspool.tile([128, CH], F32, name="mm", tag="mm", bufs=2)
                                for i in range(NDT):
                                    nc.tensor.matmul(
                                        ps[:],
                                        kT[i][:, ts(k, 128)],
                                        qT[i][ch][:],
                                        start=(i == 0), stop=(i == NDT - 1),
                                    )
                                masked = (ch == 0) or (k >= 8)
                                if masked:
                                    praw = p2.tile([128, CH], F16, name="praw", tag="praw", bufs=2)
                                    nc.scalar.activation(
                                        praw[:], ps[:], AF.Exp, scale=1.0 / 32.0
                                    )
                                    msk = p2.tile([128, CH], F16, name="msk", tag="msk", bufs=2)
                                    nc.vector.tensor_scalar(
                                        out=msk[:],
                                        in0=qposB[:, ts(ch, CH)],
                                        scalar1=iota_sb[:, k : k + 1],
                                        scalar2=None,
                                        op0=ALU.is_ge,
                                    )
                                    nc.vector.tensor_tensor(
                                        out=pt[k][:], in0=praw[:], in1=msk[:],
                                        op=ALU.mult,
                                    )
                                else:
                                    nc.scalar.activation(
                                        pt[k][:], ps[:], AF.Exp, scale=1.0 / 32.0
                                    )
                                # denominator accumulation on DVE (off the PE)
                                if k == 0:
                                    nc.vector.tensor_copy(ptsum[:], pt[0][:])
                                else:
                                    nc.vector.tensor_tensor(
                                        out=ptsum[:], in0=ptsum[:], in1=pt[k][:],
                                        op=ALU.add,
                                    )

                            recip = p2.tile([1, CH], F32, name="recip", tag="recip", bufs=2)
                            recip_r = p2.tile([1, CH], F32R, name="recip_r", tag="recip_r", bufs=2)
                            recipB = p2.tile([128, CH], F32, name="recipB", tag="recipB", bufs=2)
                            aps = {}

                            def pv_mm(d_):
                                aps[d_] = pspool.tile(
                                    [128, CH], F32, name=f"at{d_ % 5}", tag=f"at{d_ % 5}", bufs=1
                                )
                                for k in range(nkt):
                                    nc.tensor.matmul(
                                        aps[d_][:],
                                        vt[k][:, ts(d_, 128)],
                                        pt[k][:],
                                        start=(k == 0), stop=(k == nkt - 1),
                                    )

                            def pv_mult(d_):
                                nc.vector.tensor_tensor(
                                    out=outT[d_][ch][:], in0=aps[d_][:], in1=recipB[:],
                                    op=ALU.mult,
                                )

                            # PE order: PV0 | reduce | PV1-3 | broadcast | PV4-7.
                            # All outT multiplies are emitted AFTER the recipB
                            # write (program order defines tile dependencies).
                            pv_mm(0)
                            dn_ps = pspool.tile([1, CH], F32, name="small", tag="small", bufs=1)
                            nc.tensor.matmul(
                                dn_ps[:], ones_col_r[:], ptsum[:], start=True, stop=True
                            )
                            nc.vector.reciprocal(recip[:], dn_ps[:])
                            nc.vector.tensor_copy(recip_r[:], recip[:])
                            for d_ in range(1, 4):
                                pv_mm(d_)
                            rb_ps = pspool.tile([128, CH], F32, name="small", tag="small", bufs=1)
                            nc.tensor.matmul(
                                rb_ps[:], ones_row[:], recip_r[:], start=True, stop=True
                            )
                            nc.scalar.activation(recipB[:], rb_ps[:], AF.Copy)
                            for d_ in range(4):
                                pv_mult(d_)
                            for d_ in range(4, NDT):
                                pv_mm(d_)
                                pv_mult(d_)

            # ---------------- P4: FFN + GELU ----------------
            with tc.tile_pool(name="p4", bufs=1) as p4:
                for mb in range(M // 512):
                    wfb = [
                        p4.tile([128, 512], BF16, name=f"wf{i}", tag=f"wf{i}", bufs=3)
                        for i in range(NDT)
                    ]
                    for i in range(NDT):
                        nc.sync.dma_start(wfb[i][:], wfT[ts(i, 128), ts(mb, 512)])
                    for mt in range(4):
                        m = mb * 4 + mt
                        psA = pspool.tile([128, CH], F32, name="mm", tag="mm", bufs=2)
                        psB = pspool.tile([128, CH], F32, name=f"at{mt % 2}", tag=f"at{mt % 2}", bufs=1)
                        for i in range(NDT):
                            nc.tensor.matmul(
                                psA[:], wfb[i][:, ts(mt, 128)], outT[i][0][:],
                                start=(i == 0), stop=(i == NDT - 1),
                            )
                            nc.tensor.matmul(
                                psB[:], wfb[i][:, ts(mt, 128)], outT[i][1][:],
                                start=(i == 0), stop=(i == NDT - 1),
                            )
                        stA = p4.tile([128, CH], BF16, name="ffstage", tag="ffstage", bufs=6)
                        nc.scalar.activation(stA[:], psA[:], AF.Gelu, bias=bf_sb[:, m : m + 1])
                        nc.sync.dma_start(ffT[ts(m, 128), ts(0, CH)], stA[:])
                        stB = p4.tile([128, CH], BF16, name="ffstage", tag="ffstage", bufs=6)
                        nc.scalar.activation(stB[:], psB[:], AF.Gelu, bias=bf_sb[:, m : m + 1])
                        nc.sync.dma_start(ffT[ts(m, 128), ts(1, CH)], stB[:])

    nc.compile()
    return nc


def _get_program():
    global _PROGRAM
    if _PROGRAM is None:
        _PROGRAM = _build_program()
    return _PROGRAM


def _owned_ranges(core):
    """(a0, b0): start rows of the two 512-token chunks core owns."""
    half = core % 2
    if half == 0:
        return 0, 3 * CH  # chunks 0, 3
    return CH, 2 * CH  # chunks 1, 2


def _make_in_maps(x, Wq, bq, Wk, bk, Wv, bv, Wo, bo, Wf, bf):
    import ml_dtypes

    f32 = np.float32
    bf16 = ml_dtypes.bfloat16
    f64 = np.float64
    wqT = np.ascontiguousarray(Wq.T, dtype=bf16)
    wkT = np.ascontiguousarray(Wk.T, dtype=bf16)
    Wvp = (np.asarray(Wo, f64) @ np.asarray(Wv, f64))  # folded V' weight
    wvT = np.ascontiguousarray(Wvp.T.astype(f32), dtype=bf16)
    wfT = np.ascontiguousarray(Wf.T, dtype=bf16)
    # bo2 = Wo@bv + bo folded into the FFN bias: bf2 = Wf@bo2 + bf
    bo2 = np.asarray(Wo, f64) @ np.asarray(bv, f64) + np.asarray(bo, f64)
    bf2 = np.asarray(Wf, f64) @ bo2 + np.asarray(bf, f64)
    bfT = np.ascontiguousarray(bf2.astype(f32).reshape(M // 128, 128).T)
    iota = (
        np.arange(128, dtype=f32)[:, None]
        + 128.0 * np.arange(S // 128, dtype=f32)[None, :]
    )
    shared = {
        "wqT": wqT, "wkT": wkT, "wvT": wvT, "wfT": wfT,
        "bq": np.ascontiguousarray(bq.reshape(D // 128, 128).T, dtype=f32),
        "bk": np.ascontiguousarray(bk.reshape(D // 128, 128).T, dtype=f32),
        "bfT": bfT,
        "iota_kt": np.ascontiguousarray(iota),
    }
    in_maps = []
    for core in range(N_CORES):
        b = core // 2
        a0, b0 = _owned_ranges(core)
        xTb = np.ascontiguousarray(x[b].T, dtype=bf16)  # [D, S]
        xoT = np.ascontiguousarray(
            np.concatenate([xTb[:, a0 : a0 + CH], xTb[:, b0 : b0 + CH]], axis=1)
        )
        qp = np.concatenate(
            [np.arange(a0, a0 + CH), np.arange(b0, b0 + CH)]
        ).astype(f32)[None, :]
        in_maps.append(
            {**shared, "xT": xTb, "xoT": xoT, "qpos": np.ascontiguousarray(qp)}
        )
    return in_maps


def _run(inputs, trace=False, trace_cores=None, tmpdir=None):
    import sys

    if "/opt/trn_rl_repo" not in sys.path:
        sys.path.insert(0, "/opt/trn_rl_repo")
    from concourse.bass_utils import run_bass_kernel_spmd

    nc = _get_program()
    in_maps = _make_in_maps(**inputs)
    res = run_bass_kernel_spmd(
        nc, in_maps, list(range(N_CORES)), trace=trace, trace_cores=trace_cores,
        tmpdir=tmpdir,
    )
    out = np.empty((B, S, M), dtype=np.float32)
    for core in range(N_CORES):
        b = core // 2
        a0, b0 = _owned_ranges(core)
        ffT = np.asarray(res.results[core]["ffT"], dtype=np.float32)  # [M, 1024]
        out[b, a0 : a0 + CH] = ffT[:, :CH].T
        out[b, b0 : b0 + CH] = ffT[:, CH:].T
    return out, res


def kernel(**inputs):
    out, _ = _run(inputs)
    return out


# revision 6
# speedup vs baseline: 1.2694x; 1.0106x over previous
"""Decoder block (single-head causal attention + GELU FFN) on 8 TRN2 NeuronCores.

Sharding: pure data parallel, no collectives. Core c handles batch b = c//2 and
1024 query tokens of that batch, chosen as two 512-token chunks that balance the
causal-attention workload:
  even cores (half 0): chunks 0 and 3  (rows    0:512  and 1536:2048)
  odd  cores (half 1): chunks 1 and 2  (rows  512:1024 and 1024:1536)
The SPMD program is identical on every core; all per-core differences are data
(which tokens are in xoT, qpos values that drive on-chip causal-mask generation).

v4:
  - All matmul operands bf16 (fp32 PSUM accumulation); DMA and SBUF halved.
  - Wo folded into Wv on the host (softmax rows sum to 1), its bias chain
    folded into the FFN bias. The output-projection phase disappears.
  - Single x stream computes K and V' per token-block pair; paired matmul
    chains share the stationary operand (wk across the tb pair, xblk across
    the wv halves, wq across the q chunks, wf across the two out chunks).
  - V' stays in SBUF; softmax denominator accumulated on the vector engine;
    reciprocal broadcast matmuls scheduled between PV chains (after the
    recipB write — program order defines tile dependencies).
  - ffT output in bf16 (host casts back to f32).
"""

import numpy as np

D = 1024  # model dim
S = 2048  # sequence length
B = 4  # batch
M = 4096  # FFN dim
CH = 512  # q chunk (slot) size
NDT = D // 128  # 8 d-tiles
N_CORES = 8

_PROGRAM = None  # cached compiled program


def _build_program():
    import sys

    if "/opt/trn_rl_repo" not in sys.path:
        sys.path.insert(0, "/opt/trn_rl_repo")
    import concourse.bass as bass
    import concourse.tile as tile
    import concourse.mybir as mybir
    from concourse import bacc
    from concourse.bass import ts

    dt = mybir.dt
    AF = mybir.ActivationFunctionType
    ALU = mybir.AluOpType
    F32, BF16, F32R, F16 = dt.float32, dt.bfloat16, dt.float32r, dt.float16

    nc = bacc.Bacc("TRN2", target_bir_lowering=False, debug=False)

    # ---------------- DRAM I/O ----------------
    xT = nc.dram_tensor("xT", [D, S], BF16, kind="ExternalInput").ap()
    xoT = nc.dram_tensor("xoT", [D, 2 * CH], BF16, kind="ExternalInput").ap()
    wqT = nc.dram_tensor("wqT", [D, D], BF16, kind="ExternalInput").ap()
    wkT = nc.dram_tensor("wkT", [D, D], BF16, kind="ExternalInput").ap()
    wvT = nc.dram_tensor("wvT", [D, D], BF16, kind="ExternalInput").ap()  # (Wo@Wv).T
    wfT = nc.dram_tensor("wfT", [D, M], BF16, kind="ExternalInput").ap()
    bq = nc.dram_tensor("bq", [128, D // 128], F32, kind="ExternalInput").ap()
    bk = nc.dram_tensor("bk", [128, D // 128], F32, kind="ExternalInput").ap()
    bfT = nc.dram_tensor("bfT", [128, M // 128], F32, kind="ExternalInput").ap()
    qpos = nc.dram_tensor("qpos", [1, 2 * CH], F32R, kind="ExternalInput").ap()
    iota_kt = nc.dram_tensor("iota_kt", [128, S // 128], F32, kind="ExternalInput").ap()
    ffT = nc.dram_tensor("ffT", [M, 2 * CH], BF16, kind="ExternalOutput").ap()

    NKT = [8, 16]  # k-tiles per slot (slotA: k<1024, slotB: k<2048)

    with tile.TileContext(nc) as tc:
        with (
            tc.tile_pool(name="const", bufs=1) as cpool,
            tc.tile_pool(name="psum", bufs=1, space="PSUM") as pspool,
        ):
            # ---------------- constants ----------------
            ones_col_r = cpool.tile([128, 1], F32R, name="ones_col_r", tag="ones_col_r")
            ones_row_f = cpool.tile([1, 128], F32, name="ones_row_f", tag="ones_row_f")
            nc.vector.memset(ones_row_f[:], 1.0)
            ones_row = cpool.tile([1, 128], F32R, name="ones_row", tag="ones_row")
            nc.vector.tensor_copy(ones_row[:], ones_row_f[:])
            ones_col_f = cpool.tile([128, 1], F32, name="ones_col_f", tag="ones_col_f")
            nc.vector.memset(ones_col_f[:], 1.0)
            nc.vector.tensor_copy(ones_col_r[:], ones_col_f[:])
            iota_sb = cpool.tile([128, S // 128], F32, name="iota", tag="iota")
            nc.sync.dma_start(iota_sb[:], iota_kt[:])
            bq_sb = cpool.tile([128, D // 128], F32, name="bq", tag="bq")
            nc.sync.dma_start(bq_sb[:], bq[:])
            bk_sb = cpool.tile([128, D // 128], F32, name="bk", tag="bk")
            nc.sync.dma_start(bk_sb[:], bk[:])
            bf_sb = cpool.tile([128, M // 128], F32, name="bf", tag="bf")
            nc.sync.dma_start(bf_sb[:], bfT[:])
            qpos_row = cpool.tile([1, 2 * CH], F32R, name="qpos_row", tag="qpos_row")
            nc.sync.dma_start(qpos_row[:], qpos[:])

            # broadcast qpos to 128 partitions via ones outer-product
            qposB = cpool.tile([128, 2 * CH], F32, name="qposB", tag="qposB")
            for i in range(2 * CH // 512):
                bc_ps = pspool.tile([128, 512], F32, name="small", tag="small", bufs=1)
                nc.tensor.matmul(
                    bc_ps[:], ones_row[:], qpos_row[:, ts(i, 512)],
                    start=True, stop=True,
                )
                nc.scalar.activation(qposB[:, ts(i, 512)], bc_ps[:], AF.Copy)

            # attention output (normalized, bf16) — lives P2..P4
            outT = [
                [cpool.tile([128, CH], BF16, name=f"oT{dt_}_{c}", tag=f"oT{dt_}_{c}") for c in range(2)]
                for dt_ in range(NDT)
            ]

            with tc.tile_pool(name="ktpool", bufs=1) as ktp:
                # K^T (bf16) and V' (fp16, token-major) stay resident through P2
                kT = [ktp.tile([128, S], BF16, name=f"kT{i}", tag=f"kT{i}") for i in range(NDT)]
                vt = [ktp.tile([128, D], F16, name=f"v{k}", tag=f"v{k}") for k in range(16)]

                with tc.tile_pool(name="qtpool", bufs=1) as qtp:
                    qT = [
                        [qtp.tile([128, CH], BF16, name=f"qT{dt_}_{qb}", tag=f"qT{dt_}_{qb}") for qb in range(2)]
                        for dt_ in range(NDT)
                    ]

                    # ---------------- P1: K / V' / Q projections ----------------
                    with tc.tile_pool(name="p1", bufs=1) as p1:
                        wk_sb = [p1.tile([128, D], BF16, name=f"wk{i}", tag=f"wk{i}") for i in range(NDT)]
                        wv_sb = [p1.tile([128, D], BF16, name=f"wv{i}", tag=f"wv{i}") for i in range(NDT)]
                        wq_sb = [p1.tile([128, D], BF16, name=f"wq{i}", tag=f"wq{i}") for i in range(NDT)]
                        xo_sb = [p1.tile([128, 2 * CH], BF16, name=f"xo{i}", tag=f"xo{i}") for i in range(NDT)]
                        # startup order: wk h0, then first x block, then wk h1
                        for i in range(NDT):
                            nc.sync.dma_start(wk_sb[i][:, ts(0, 512)], wkT[ts(i, 128), ts(0, 512)])
                        # merged K+V pass over one x stream, tb pairs
                        for p in range(2):
                            xblk = [
                                [
                                    p1.tile([128, 512], BF16, name=f"xa{i}_{j}", tag=f"xa{i}_{j}", bufs=2)
                                    for i in range(NDT)
                                ]
                                for j in range(2)
                            ]
                            for i in range(NDT):
                                nc.sync.dma_start(xblk[0][i][:], xT[ts(i, 128), ts(2 * p, 512)])
                            for i in range(NDT):
                                nc.sync.dma_start(xblk[1][i][:], xT[ts(i, 128), ts(2 * p + 1, 512)])
                            if p == 0:
                                for i in range(NDT):
                                    nc.sync.dma_start(
                                        wk_sb[i][:, ts(1, 512)], wkT[ts(i, 128), ts(1, 512)]
                                    )
                            # stagger remaining loads behind the x stream
                            if p == 0:
                                for h in range(2):
                                    for i in range(NDT):
                                        nc.sync.dma_start(
                                            wv_sb[i][:, ts(h, 512)], wvT[ts(i, 128), ts(h, 512)]
                                        )
                            else:
                                for h in range(2):
                                    for i in range(NDT):
                                        nc.sync.dma_start(
                                            wq_sb[i][:, ts(h, 512)], wqT[ts(i, 128), ts(h, 512)]
                                        )
                                for h in range(2):
                                    for i in range(NDT):
                                        nc.sync.dma_start(
                                            xo_sb[i][:, ts(h, 512)], xoT[ts(i, 128), ts(h, 512)]
                                        )
                            # K chains: stationary wk slice shared across the tb pair
                            for ot in range(NDT):
                                psA = pspool.tile([128, 512], F32, name="mm", tag="mm", bufs=2)
                                psB = pspool.tile([128, 512], F32, name=f"at{ot % 2}", tag=f"at{ot % 2}", bufs=1)
                                for i in range(NDT):
                                    nc.tensor.matmul(
                                        psA[:], wk_sb[i][:, ts(ot, 128)], xblk[0][i][:],
                                        start=(i == 0), stop=(i == NDT - 1),
                                    )
                                    nc.tensor.matmul(
                                        psB[:], wk_sb[i][:, ts(ot, 128)], xblk[1][i][:],
                                        start=(i == 0), stop=(i == NDT - 1),
                                    )
                                nc.scalar.activation(
                                    kT[ot][:, ts(2 * p, 512)], psA[:], AF.Identity,
                                    bias=bk_sb[:, ot : ot + 1],
                                )
                                nc.scalar.activation(
                                    kT[ot][:, ts(2 * p + 1, 512)], psB[:], AF.Identity,
                                    bias=bk_sb[:, ot : ot + 1],
                                )
                            # V chains: stationary x token-tile shared across wv halves
                            for j in range(2):
                                tb = 2 * p + j
                                for tt in range(4):
                                    k = tb * 4 + tt
                                    psA = pspool.tile([128, 512], F32, name="mm", tag="mm", bufs=2)
                                    psB = pspool.tile([128, 512], F32, name=f"at{tt % 2}", tag=f"at{tt % 2}", bufs=1)
                                    for i in range(NDT):
                                        nc.tensor.matmul(
                                            psA[:], xblk[j][i][:, ts(tt, 128)], wv_sb[i][:, ts(0, 512)],
                                            start=(i == 0), stop=(i == NDT - 1),
                                        )
                                        nc.tensor.matmul(
                                            psB[:], xblk[j][i][:, ts(tt, 128)], wv_sb[i][:, ts(1, 512)],
                                            start=(i == 0), stop=(i == NDT - 1),
                                        )
                                    nc.scalar.activation(vt[k][:, ts(0, 512)], psA[:], AF.Copy)
                                    nc.scalar.activation(vt[k][:, ts(1, 512)], psB[:], AF.Copy)
                        # Q pass: stationary wq slice shared across the q chunks
                        for ot in range(NDT):
                            psA = pspool.tile([128, 512], F32, name="mm", tag="mm", bufs=2)
                            psB = pspool.tile([128, 512], F32, name=f"at{ot % 2}", tag=f"at{ot % 2}", bufs=1)
                            for i in range(NDT):
                                nc.tensor.matmul(
                                    psA[:], wq_sb[i][:, ts(ot, 128)], xo_sb[i][:, ts(0, 512)],
                                    start=(i == 0), stop=(i == NDT - 1),
                                )
                                nc.tensor.matmul(
                                    psB[:], wq_sb[i][:, ts(ot, 128)], xo_sb[i][:, ts(1, 512)],
                                    start=(i == 0), stop=(i == NDT - 1),
                                )
                            nc.scalar.activation(
                                qT[ot][0][:], psA[:], AF.Identity, bias=bq_sb[:, ot : ot + 1]
                            )
                            nc.scalar.activation(
                                qT[ot][1][:], psB[:], AF.Identity, bias=bq_sb[:, ot : ot + 1]
                            )

                    # ---------------- P2: attention ----------------
                    with tc.tile_pool(name="p2", bufs=1) as p2:
                        for ch in range(2):
                            nkt = NKT[ch]
                            pt = [
                                p2.tile([128, CH], F16, name=f"pt{k}", tag=f"pt{k}", bufs=1)
                                for k in range(nkt)
                            ]
                            ptsum = p2.tile([128, CH], F32R, name="ptsum", tag="ptsum", bufs=2)
                            for k in range(nkt):
                                ps = pspool.tile([128, CH], F32, name="mm", tag="mm", bufs=2)
                                for i in range(NDT):
                                    nc.tensor.matmul(
                                        ps[:],
                                        kT[i][:, ts(k, 128)],
                                        qT[i][ch][:],
                                        start=(i == 0), stop=(i == NDT - 1),
                                    )
                                masked = (ch == 0) or (k >= 8)
                                if masked:
                                    praw = p2.tile([128, CH], F16, name="praw", tag="praw", bufs=2)
                                    nc.scalar.activation(
                                        praw[:], ps[:], AF.Exp, scale=1.0 / 32.0
                                    )
                                    msk = p2.tile([128, CH], F16, name="msk", tag="msk", bufs=2)
                                    nc.vector.tensor_scalar(
                                        out=msk[:],
                                        in0=qposB[:, ts(ch, CH)],
                                        scalar1=iota_sb[:, k : k + 1],
                                        scalar2=None,
                                        op0=ALU.is_ge,
                                    )
                                    nc.vector.tensor_tensor(
                                        out=pt[k][:], in0=praw[:], in1=msk[:],
                                        op=ALU.mult,
                                    )
                                else:
                                    nc.scalar.activation(
                                        pt[k][:], ps[:], AF.Exp, scale=1.0 / 32.0
                                    )
                                # denominator accumulation on DVE (off the PE)
                                if k == 0:
                                    nc.vector.tensor_copy(ptsum[:], pt[0][:])
                                else:
                                    nc.vector.tensor_tensor(
                                        out=ptsum[:], in0=ptsum[:], in1=pt[k][:],
                                        op=ALU.add,
                                    )

                            recip = p2.tile([1, CH], F32, name="recip", tag="recip", bufs=2)
                            recip_r = p2.tile([1, CH], F32R, name="recip_r", tag="recip_r", bufs=2)
                            recipB = p2.tile([128, CH], F32, name="recipB", tag="recipB", bufs=2)
                            aps = {}

                            def pv_mm(d_):
                                aps[d_] = pspool.tile(
                                    [128, CH], F32, name=f"at{d_ % 5}", tag=f"at{d_ % 5}", bufs=1
                                )
                                for k in range(nkt):
                                    nc.tensor.matmul(
                                        aps[d_][:],
                                        vt[k][:, ts(d_, 128)],
                                        pt[k][:],
                                        start=(k == 0), stop=(k == nkt - 1),
                                    )

                            def pv_mult(d_):
                                nc.vector.tensor_tensor(
                                    out=outT[d_][ch][:], in0=aps[d_][:], in1=recipB[:],
                                    op=ALU.mult,
                                )

                            # PE order: PV0 | reduce | PV1-3 | broadcast | PV4-7.
                            # All outT multiplies are emitted AFTER the recipB
                            # write (program order defines tile dependencies).
                            pv_mm(0)
                            dn_ps = pspool.tile([1, CH], F32, name="small", tag="small", bufs=1)
                            nc.tensor.matmul(
                                dn_ps[:], ones_col_r[:], ptsum[:], start=True, stop=True
                            )
                            nc.vector.reciprocal(recip[:], dn_ps[:])
                            nc.vector.tensor_copy(recip_r[:], recip[:])
                            for d_ in range(1, 4):
                                pv_mm(d_)
                            rb_ps = pspool.tile([128, CH], F32, name="small", tag="small", bufs=1)
                            nc.tensor.matmul(
                                rb_ps[:], ones_row[:], recip_r[:], start=True, stop=True
                            )
                            nc.scalar.activation(recipB[:], rb_ps[:], AF.Copy)
                            for d_ in range(4):
                                pv_mult(d_)
                            for d_ in range(4, NDT):
                                pv_mm(d_)
                                pv_mult(d_)

            # ---------------- P4: FFN + GELU ----------------
            with tc.tile_pool(name="p4", bufs=1) as p4:
                for mb in range(M // 512):
                    wfb = [
                        p4.tile([128, 512], BF16, name=f"wf{i}", tag=f"wf{i}", bufs=3)
                        for i in range(NDT)
                    ]
                    for i in range(NDT):
                        nc.sync.dma_start(wfb[i][:], wfT[ts(i, 128), ts(mb, 512)])
                    for mt in range(4):
                        m = mb * 4 + mt
                        psA = pspool.tile([128, CH], F32, name="mm", tag="mm", bufs=2)
                        psB = pspool.tile([128, CH], F32, name=f"at{mt % 2}", tag=f"at{mt % 2}", bufs=1)
                        if mb == 0 and mt == 0:
                            # unpaired: ch1's last outT multiplies may still be
                            # in flight right after P2 — don't gate the first
                            # chain on them
                            for i in range(NDT):
                                nc.tensor.matmul(
                                    psA[:], wfb[i][:, ts(mt, 128)], outT[i][0][:],
                                    start=(i == 0), stop=(i == NDT - 1),
                                )
                            for i in range(NDT):
                                nc.tensor.matmul(
                                    psB[:], wfb[i][:, ts(mt, 128)], outT[i][1][:],
                                    start=(i == 0), stop=(i == NDT - 1),
                                )
                        else:
                            for i in range(NDT):
                                nc.tensor.matmul(
                                    psA[:], wfb[i][:, ts(mt, 128)], outT[i][0][:],
                                    start=(i == 0), stop=(i == NDT - 1),
                                )
                                nc.tensor.matmul(
                                    psB[:], wfb[i][:, ts(mt, 128)], outT[i][1][:],
                                    start=(i == 0), stop=(i == NDT - 1),
                                )
                        st = p4.tile([128, 2 * CH], BF16, name="ffstage", tag="ffstage", bufs=4)
                        nc.scalar.activation(st[:, ts(0, CH)], psA[:], AF.Gelu, bias=bf_sb[:, m : m + 1])
                        nc.scalar.activation(st[:, ts(1, CH)], psB[:], AF.Gelu, bias=bf_sb[:, m : m + 1])
                        nc.sync.dma_start(ffT[ts(m, 128), :], st[:])

    nc.compile()
    return nc


def _get_program():
    global _PROGRAM
    if _PROGRAM is None:
        _PROGRAM = _build_program()
    return _PROGRAM


def _owned_ranges(core):
    """(a0, b0): start rows of the two 512-token chunks core owns."""
    half = core % 2
    if half == 0:
        return 0, 3 * CH  # chunks 0, 3
    return CH, 2 * CH  # chunks 1, 2


def _make_in_maps(x, Wq, bq, Wk, bk, Wv, bv, Wo, bo, Wf, bf):
    import ml_dtypes

    f32 = np.float32
    bf16 = ml_dtypes.bfloat16
    f64 = np.float64
    wqT = np.ascontiguousarray(Wq.T, dtype=bf16)
    wkT = np.ascontiguousarray(Wk.T, dtype=bf16)
    Wvp = (np.asarray(Wo, f64) @ np.asarray(Wv, f64))  # folded V' weight
    wvT = np.ascontiguousarray(Wvp.T.astype(f32), dtype=bf16)
    wfT = np.ascontiguousarray(Wf.T, dtype=bf16)
    # bo2 = Wo@bv + bo folded into the FFN bias: bf2 = Wf@bo2 + bf
    bo2 = np.asarray(Wo, f64) @ np.asarray(bv, f64) + np.asarray(bo, f64)
    bf2 = np.asarray(Wf, f64) @ bo2 + np.asarray(bf, f64)
    bfT = np.ascontiguousarray(bf2.astype(f32).reshape(M // 128, 128).T)
    iota = (
        np.arange(128, dtype=f32)[:, None]
        + 128.0 * np.arange(S // 128, dtype=f32)[None, :]
    )
    shared = {
        "wqT": wqT, "wkT": wkT, "wvT": wvT, "wfT": wfT,
        "bq": np.ascontiguousarray(bq.reshape(D // 128, 128).T, dtype=f32),
        "bk": np.ascontiguousarray(bk.reshape(D // 128, 128).T, dtype=f32),
        "bfT": bfT,
        "iota_kt": np.ascontiguousarray(iota),
    }
    in_maps = []
    for core in range(N_CORES):
        b = core // 2
        a0, b0 = _owned_ranges(core)
        xTb = np.ascontiguousarray(x[b].T, dtype=bf16)  # [D, S]
        xoT = np.ascontiguousarray(
            np.concatenate([xTb[:, a0 : a0 + CH], xTb[:, b0 : b0 + CH]], axis=1)
        )
        qp = np.concatenate(
            [np.arange(a0, a0 + CH), np.arange(b0, b0 + CH)]
        ).astype(f32)[None, :]
        in_maps.append(
            {**shared, "xT": xTb, "xoT": xoT, "qpos": np.ascontiguousarray(qp)}
        )
    return in_maps


def _run(inputs, trace=False, trace_cores=None, tmpdir=None):
    import sys

    if "/opt/trn_rl_repo" not in sys.path:
        sys.path.insert(0, "/opt/trn_rl_repo")
    from concourse.bass_utils import run_bass_kernel_spmd

    nc = _get_program()
    in_maps = _make_in_maps(**inputs)
    res = run_bass_kernel_spmd(
        nc, in_maps, list(range(N_CORES)), trace=trace, trace_cores=trace_cores,
        tmpdir=tmpdir,
    )
    out = np.empty((B, S, M), dtype=np.float32)
    for core in range(N_CORES):
        b = core // 2
        a0, b0 = _owned_ranges(core)
        ffT = np.asarray(res.results[core]["ffT"], dtype=np.float32)  # [M, 1024]
        out[b, a0 : a0 + CH] = ffT[:, :CH].T
        out[b, b0 : b0 + CH] = ffT[:, CH:].T
    return out, res


def kernel(**inputs):
    out, _ = _run(inputs)
    return out
